# revision 14
# baseline (speedup 1.0000x reference)
"""LinkPredictor similarity kernel for 8 Trainium2 NeuronCores.

reference:
    sims = E @ E.T               # [16384, 16384], E = [16384, 512] fp32
    m, M = sims.min(), sims.max()
    sims = (sims - m) / (M - m + 1e-7)
    out  = sims[row_idx, col_idx]     # block-diag strict-upper-tri gather

Only the 128 diagonal [128,128] graph blocks are ever gathered, but the
global min needs every entry of sims. Two mathematical shortcuts:
  * sims is symmetric -> min over the block upper triangle suffices.
  * By Cauchy-Schwarz, s_ij <= |e_i||e_j| <= max_k |e_k|^2 = max diag,
    so the global max is exactly the max diagonal entry.

Distribution: 16 half-slabs of 1024 rows; core c owns half-slabs
{c, 15-c} and the 17 upper-triangle [1024,1024] blocks whose row
half-slab is one of those (every core gets exactly 17 blocks). The host
packs each core's block operands so the two DIAGONAL slab-blocks are
always at positions 0 and 1 -- the program is identical across cores
(SPMD) while the content differs. For those two blocks only the
upper-triangle pairs are computed (4.5 of 8 pair-equivalents), and the
leading [128,128] of each partial pair IS that graph's diagonal block,
which is copied out directly -- no separate bf16 diagonal pass at all.

The sweep runs fp8e4 (e4m3) matmuls in DoubleRow perf mode (K=256 per
instruction, ~2x bf16 PE throughput, measured at the fp8 roofline):
per [1024,1024] block, 8 two-bank PSUM pairs [128,1024] are each
filled by 4 matmuls. The running elementwise min is split to keep both
elementwise engines below the TensorE pace: per 8 pairs, 6 go
ScalarE-copy(fp16) -> VectorE running tensor_tensor min (two
alternating accumulators to break the RAW chain), 2 go VectorE
tensor_reduce-min directly on fp32 PSUM into independent slot columns.
All min state (two fp16 accumulators + slots) is DMAd out raw and
reduced on the host. Host-simulated numerics: total pipeline rel err
~6e-3 vs the 2e-2 gate.

Streams are fully resident in SBUF (17 x 8KB/partition) with every
input DMA issued up front on the SP HW-DGE ring (block 0's operands
partly on the ACT ring for a parallel fast start), so no DMA ever
queues behind compute.
"""

import numpy as np
import ml_dtypes

N_GRAPHS = 128
G = 128
D = 512
N = N_GRAPHS * G          # 16384
EPS = 1e-7
NCORES = 8
HS = 1024                 # half-slab rows
NHS = N // HS             # 16 half-slabs
NBLK = 17                 # triangle blocks per core
KC = D // 128             # 4 contraction chunks of 128
MT = HS // 128            # 8 m-tiles per block
GPC = 16                  # graphs per core

_CACHED = {}
LAST_RESULTS = None       # BassKernelResults of the most recent run

# per-8-pair routing for full blocks: 'a' = scalar->fp16->vector TT min,
# 'c' = vector tensor_reduce min direct on fp32 PSUM into a slot column
ROUTE = "aacaacaa"
NSLOTS = 64               # 2x15 full + 16 diag-partial = 46 used


def _build_program():
    import concourse.bacc as bacc
    import concourse.mybir as mybir
    from concourse.tile import TileContext

    f32 = mybir.dt.float32
    f16 = mybir.dt.float16
    f8 = mybir.dt.float8e4
    DR = mybir.MatmulPerfMode.DoubleRow
    MIN = mybir.AluOpType.min

    nc = bacc.Bacc(target_bir_lowering=False)
    # per-partition-contiguous packing: [block, partition, d1, col];
    # cols [0:1024] = stationary slab, [1024:2048] = moving slab
    lr = nc.declare_dram_parameter("lr", [NBLK, 128, KC, 2 * HS], f8, isOutput=False)
    diag_out = nc.declare_dram_parameter("diag_out", [GPC, G, G], f32, isOutput=True)
    rm0_out = nc.declare_dram_parameter("rm0", [128, 1024], f16, isOutput=True)
    rm1_out = nc.declare_dram_parameter("rm1", [128, 1024], f16, isOutput=True)
    slots_out = nc.declare_dram_parameter("slots_o", [128, NSLOTS], f32, isOutput=True)

    with TileContext(nc) as tc:
        with (
            tc.tile_pool(name="stream", bufs=1) as stream,
            tc.tile_pool(name="small", bufs=4) as small,
            tc.tile_pool(name="cpp", bufs=3) as cpp,
            tc.tile_pool(name="acc", bufs=1) as accp,
            tc.tile_pool(name="ps", bufs=4, space="PSUM") as ps,
        ):
            run_min = [
                accp.tile([128, 1024], f16, tag=f"run_min{i}", name=f"run_min{i}")
                for i in range(2)
            ]
            slots = accp.tile([128, NSLOTS], f32, tag="slots")
            nc.vector.memset(run_min[0][:], 60000.0)
            nc.vector.memset(run_min[1][:], 60000.0)
            nc.vector.memset(slots[:], 3.0e38)

            # fully-resident streams, all input DMA issued up front
            lrs = []
            for b in range(NBLK):
                lrt = stream.tile(
                    [128, KC, 2 * HS], f8, tag=f"lr{b}", name=f"lr{b}"
                )
                lrs.append(lrt)
                if b < 2:
                    # diagonal blocks read only the stationary half
                    if b == 0:
                        nc.scalar.dma_start(
                            out=lrt[:, :, 0:512], in_=lr[b][:, :, 0:512]
                        )
                        nc.sync.dma_start(
                            out=lrt[:, :, 512:HS], in_=lr[b][:, :, 512:HS]
                        )
                    else:
                        nc.sync.dma_start(out=lrt[:, :, 0:HS], in_=lr[b][:, :, 0:HS])
                else:
                    nc.sync.dma_start(out=lrt[:], in_=lr[b])

            na = 0
            nslot = 0
            for b in range(NBLK):
                lrt = lrs[b]
                if b < 2:
                    # diagonal slab-block: upper-triangle pairs only; moving
                    # operand comes from the stationary half (same slab).
                    for m in range(MT):
                        off = m * 128
                        w = HS - off
                        acc = ps.tile([128, 1024], f32, tag="acc")
                        for n0 in range(0, w, 512):
                            nw = min(512, w - n0)
                            for k2 in range(2):
                                nc.tensor.matmul(
                                    acc[:, n0 : n0 + nw],
                                    lrt[:, 2 * k2 : 2 * k2 + 2, off : off + 128],
                                    lrt[
                                        :, 2 * k2 : 2 * k2 + 2,
                                        off + n0 : off + n0 + nw,
                                    ],
                                    start=(k2 == 0), stop=(k2 == 1),
                                    perf_mode=DR,
                                )
                        # leading [128,128] is graph (b*8+m)'s diagonal block
                        dcp = small.tile([128, G], f32, tag="dcp")
                        nc.scalar.copy(dcp[:], acc[:, 0:G])
                        nc.sync.dma_start(out=diag_out[b * MT + m], in_=dcp[:])
                        nc.vector.tensor_reduce(
                            slots[:, nslot : nslot + 1], acc[:, 0:w],
                            mybir.AxisListType.X, MIN,
                        )
                        nslot += 1
                else:
                    for m in range(MT):
                        acc = ps.tile([128, 1024], f32, tag="acc")
                        for n in range(2):
                            for k2 in range(2):
                                nc.tensor.matmul(
                                    acc[:, n * 512 : (n + 1) * 512],
                                    lrt[
                                        :, 2 * k2 : 2 * k2 + 2,
                                        m * 128 : (m + 1) * 128,
                                    ],
                                    lrt[
                                        :, 2 * k2 : 2 * k2 + 2,
                                        HS + n * 512 : HS + (n + 1) * 512,
                                    ],
                                    start=(k2 == 0), stop=(k2 == 1),
                                    perf_mode=DR,
                                )
                        if ROUTE[m] == "a":
                            cp = cpp.tile([128, 1024], f16, tag="cpv")
                            nc.scalar.copy(cp[:], acc[:])
                            rm = run_min[na % 2]
                            na += 1
                            nc.vector.tensor_tensor(rm[:], rm[:], cp[:], MIN)
                        else:
                            nc.vector.tensor_reduce(
                                slots[:, nslot : nslot + 1], acc[:],
                                mybir.AxisListType.X, MIN,
                            )
                            nslot += 1

            nc.sync.dma_start(out=rm0_out[:], in_=run_min[0][:])
            nc.sync.dma_start(out=rm1_out[:], in_=run_min[1][:])
            nc.sync.dma_start(out=slots_out[:], in_=slots[:])

    nc.finalize()
    return nc


def _core_items(c: int):
    """Block list with the two diagonal blocks first (positions 0, 1)."""
    a, bb = c, NHS - 1 - c
    items = [(a, a), (bb, bb)]
    items += [(a, j) for j in range(a + 1, NHS)]
    items += [(bb, j) for j in range(bb + 1, NHS)]
    assert len(items) == NBLK
    return items


def kernel(embeddings, row_idx, col_idx):
    global LAST_RESULTS
    from concourse.bass_utils import run_bass_kernel_spmd

    emb = np.asarray(embeddings, dtype=np.float32)
    row_idx = np.asarray(row_idx)
    col_idx = np.asarray(col_idx)

    if "nc" not in _CACHED:
        _CACHED["nc"] = _build_program()
    nc = _CACHED["nc"]

    eT = np.ascontiguousarray(emb.T)                       # [512, 16384] fp32
    e8 = eT.astype(ml_dtypes.float8_e4m3)                  # e4m3, RTNE
    # [slab, partition, d1, col]: per-partition-contiguous 4KB lines
    p8 = np.ascontiguousarray(
        e8.reshape(KC, 128, NHS, HS).transpose(2, 1, 0, 3)
    )

    in_maps = []
    for c in range(NCORES):
        items = _core_items(c)
        lr = np.concatenate(
            [p8[[i for i, _ in items]], p8[[j for _, j in items]]], axis=3
        )
        in_maps.append({"lr": lr})

    res = run_bass_kernel_spmd(nc, in_maps, list(range(NCORES)))
    LAST_RESULTS = res

    m = min(
        min(
            np.asarray(r["rm0"], np.float32).min(),
            np.asarray(r["rm1"], np.float32).min(),
            r["slots_o"].min(),
        )
        for r in res.results
    )

    blocks = np.empty((N_GRAPHS, G, G), np.float32)
    gph = HS // G  # graphs per half-slab = 8
    for c in range(NCORES):
        rows = [c, NHS - 1 - c]
        gids = [i * gph + k for i in rows for k in range(gph)]
        for idx, g in enumerate(gids):
            blocks[g] = res.results[c]["diag_out"][idx]

    M = np.einsum("gii->gi", blocks).max()                 # global max (Cauchy-Schwarz)

    norm = (blocks - m) / (M - m + EPS)
    r = row_idx.astype(np.int64)
    cc = col_idx.astype(np.int64)
    out = norm[r >> 7, r & 127, cc & 127].astype(np.float32)
    return out


# revision 15
# speedup vs baseline: 1.2973x; 1.2973x over previous
"""LinkPredictor similarity kernel for 8 Trainium2 NeuronCores.

reference:
    sims = E @ E.T               # [16384, 16384], E = [16384, 512] fp32
    m, M = sims.min(), sims.max()
    sims = (sims - m) / (M - m + 1e-7)
    out  = sims[row_idx, col_idx]     # block-diag strict-upper-tri gather

Only the 128 diagonal [128,128] graph blocks are ever gathered, but the
global min needs every entry of sims. Two mathematical shortcuts:
  * sims is symmetric -> min over the block upper triangle suffices.
  * By Cauchy-Schwarz, s_ij <= |e_i||e_j| <= max_k |e_k|^2 = max diag,
    so the global max is exactly the max diagonal entry.

Distribution: 16 half-slabs of 1024 rows; core c owns half-slabs
{c, 15-c} and the 17 upper-triangle [1024,1024] blocks whose row
half-slab is one of those (every core gets exactly 17 blocks). The host
packs each core's block operands so the two DIAGONAL slab-blocks are
always at positions 0 and 1 -- the program is identical across cores
(SPMD) while the content differs. For those two blocks only the
upper-triangle pairs are computed (4.5 of 8 pair-equivalents), and the
leading [128,128] of each partial pair IS that graph's diagonal block,
which is copied out directly -- no separate bf16 diagonal pass at all.

The sweep runs fp8e4 (e4m3) matmuls in DoubleRow perf mode (K=256 per
instruction, ~2x bf16 PE throughput, measured at the fp8 roofline):
per [1024,1024] block, 8 two-bank PSUM pairs [128,1024] are each
filled by 4 matmuls. The running elementwise min is split to keep both
elementwise engines below the TensorE pace: per 8 pairs, 6 go
ScalarE-copy(fp16) -> VectorE running tensor_tensor min (two
alternating accumulators to break the RAW chain), 2 go VectorE
tensor_reduce-min directly on fp32 PSUM into independent slot columns.
All min state (two fp16 accumulators + slots) is DMAd out raw and
reduced on the host. Host-simulated numerics: total pipeline rel err
~6e-3 vs the 2e-2 gate.

Streams are fully resident in SBUF (17 x 8KB/partition) with every
input DMA issued up front on the SP HW-DGE ring (block 0's operands
partly on the ACT ring for a parallel fast start), so no DMA ever
queues behind compute.
"""

import numpy as np
import ml_dtypes

N_GRAPHS = 128
G = 128
D = 512
N = N_GRAPHS * G          # 16384
EPS = 1e-7
NCORES = 8
HS = 1024                 # half-slab rows
NHS = N // HS             # 16 half-slabs
NBLK = 17                 # triangle blocks per core
KC = D // 128             # 4 contraction chunks of 128
MT = HS // 128            # 8 m-tiles per block
GPC = 16                  # graphs per core

_CACHED = {}
LAST_RESULTS = None       # BassKernelResults of the most recent run

# per-8-pair routing for full blocks: 'a' = scalar->fp16->vector TT min,
# 'c' = vector tensor_reduce min direct on fp32 PSUM into a slot column
ROUTE = "aacaacaa"
NSLOTS = 64               # 2x15 full + 16 diag-partial = 46 used


def _build_program():
    import concourse.bacc as bacc
    import concourse.mybir as mybir
    from concourse.tile import TileContext

    f32 = mybir.dt.float32
    f16 = mybir.dt.float16
    f8 = mybir.dt.float8e4
    DR = mybir.MatmulPerfMode.DoubleRow
    MIN = mybir.AluOpType.min

    nc = bacc.Bacc(target_bir_lowering=False)
    # per-partition-contiguous packing: [block, partition, d1, col];
    # cols [0:1024] = stationary slab, [1024:2048] = moving slab
    lr = nc.declare_dram_parameter("lr", [NBLK, 128, KC, 2 * HS], f8, isOutput=False)
    diag_out = nc.declare_dram_parameter("diag_out", [128, GPC * G], f32, isOutput=True)
    rm0_out = nc.declare_dram_parameter("rm0", [128, 1024], f16, isOutput=True)
    rm1_out = nc.declare_dram_parameter("rm1", [128, 1024], f16, isOutput=True)
    slots_out = nc.declare_dram_parameter("slots_o", [128, NSLOTS], f32, isOutput=True)

    with TileContext(nc) as tc:
        with (
            tc.tile_pool(name="stream", bufs=1) as stream,
            tc.tile_pool(name="small", bufs=4) as small,
            tc.tile_pool(name="cpp", bufs=3) as cpp,
            tc.tile_pool(name="acc", bufs=1) as accp,
            tc.tile_pool(name="ps", bufs=4, space="PSUM") as ps,
        ):
            run_min = [
                accp.tile([128, 1024], f16, tag=f"run_min{i}", name=f"run_min{i}")
                for i in range(2)
            ]
            slots = accp.tile([128, NSLOTS], f32, tag="slots")
            dall = accp.tile([128, GPC * G], f32, tag="dall")
            nc.vector.memset(run_min[0][:], 60000.0)
            nc.vector.memset(run_min[1][:], 60000.0)
            nc.vector.memset(slots[:], 3.0e38)

            # fully-resident streams, all input DMA issued up front
            lrs = []
            for b in range(NBLK):
                lrt = stream.tile(
                    [128, KC, 2 * HS], f8, tag=f"lr{b}", name=f"lr{b}"
                )
                lrs.append(lrt)
                if b < 2:
                    # diagonal blocks read only the stationary half
                    if b == 0:
                        nc.scalar.dma_start(
                            out=lrt[:, :, 0:512], in_=lr[b][:, :, 0:512]
                        )
                        nc.sync.dma_start(
                            out=lrt[:, :, 512:HS], in_=lr[b][:, :, 512:HS]
                        )
                    else:
                        nc.sync.dma_start(out=lrt[:, :, 0:HS], in_=lr[b][:, :, 0:HS])
                else:
                    nc.sync.dma_start(out=lrt[:], in_=lr[b])

            na = 0
            nslot = 0
            for b in range(NBLK):
                lrt = lrs[b]
                if b < 2:
                    # diagonal slab-block: upper-triangle pairs only; moving
                    # operand comes from the stationary half (same slab).
                    for m in range(MT):
                        off = m * 128
                        w = HS - off
                        g = b * MT + m
                        acc = ps.tile([128, 1024], f32, tag="acc")
                        for n0 in range(0, w, 512):
                            nw = min(512, w - n0)
                            for k2 in range(2):
                                nc.tensor.matmul(
                                    acc[:, n0 : n0 + nw],
                                    lrt[:, 2 * k2 : 2 * k2 + 2, off : off + 128],
                                    lrt[
                                        :, 2 * k2 : 2 * k2 + 2,
                                        off + n0 : off + n0 + nw,
                                    ],
                                    start=(k2 == 0), stop=(k2 == 1),
                                    perf_mode=DR,
                                )
                        # leading [128,128] is graph g's diagonal block; park
                        # it in the persistent dall tile (one big DMA later)
                        nc.scalar.copy(dall[:, g * G : (g + 1) * G], acc[:, 0:G])
                        if m % 2 == 0:
                            nc.vector.tensor_reduce(
                                slots[:, nslot : nslot + 1], acc[:, 0:w],
                                mybir.AxisListType.X, MIN,
                            )
                            nslot += 1
                        else:
                            cp = cpp.tile([128, 1024], f16, tag="cpv")
                            nc.scalar.copy(cp[:, 0:w], acc[:, 0:w])
                            rm = run_min[na % 2]
                            na += 1
                            nc.vector.tensor_tensor(
                                rm[:, 0:w], rm[:, 0:w], cp[:, 0:w], MIN
                            )
                    if b == 1:
                        nc.sync.dma_start(out=diag_out[:], in_=dall[:])
                else:
                    for m in range(MT):
                        acc = ps.tile([128, 1024], f32, tag="acc")
                        for n in range(2):
                            for k2 in range(2):
                                nc.tensor.matmul(
                                    acc[:, n * 512 : (n + 1) * 512],
                                    lrt[
                                        :, 2 * k2 : 2 * k2 + 2,
                                        m * 128 : (m + 1) * 128,
                                    ],
                                    lrt[
                                        :, 2 * k2 : 2 * k2 + 2,
                                        HS + n * 512 : HS + (n + 1) * 512,
                                    ],
                                    start=(k2 == 0), stop=(k2 == 1),
                                    perf_mode=DR,
                                )
                        if ROUTE[m] == "a":
                            cp = cpp.tile([128, 1024], f16, tag="cpv")
                            nc.scalar.copy(cp[:], acc[:])
                            rm = run_min[na % 2]
                            na += 1
                            nc.vector.tensor_tensor(rm[:], rm[:], cp[:], MIN)
                        else:
                            nc.vector.tensor_reduce(
                                slots[:, nslot : nslot + 1], acc[:],
                                mybir.AxisListType.X, MIN,
                            )
                            nslot += 1

            nc.sync.dma_start(out=rm0_out[:], in_=run_min[0][:])
            nc.sync.dma_start(out=rm1_out[:], in_=run_min[1][:])
            nc.sync.dma_start(out=slots_out[:], in_=slots[:])

    nc.finalize()
    return nc


def _core_items(c: int):
    """Block list with the two diagonal blocks first (positions 0, 1)."""
    a, bb = c, NHS - 1 - c
    items = [(a, a), (bb, bb)]
    items += [(a, j) for j in range(a + 1, NHS)]
    items += [(bb, j) for j in range(bb + 1, NHS)]
    assert len(items) == NBLK
    return items


def kernel(embeddings, row_idx, col_idx):
    global LAST_RESULTS
    from concourse.bass_utils import run_bass_kernel_spmd

    emb = np.asarray(embeddings, dtype=np.float32)
    row_idx = np.asarray(row_idx)
    col_idx = np.asarray(col_idx)

    if "nc" not in _CACHED:
        _CACHED["nc"] = _build_program()
    nc = _CACHED["nc"]

    eT = np.ascontiguousarray(emb.T)                       # [512, 16384] fp32
    e8 = eT.astype(ml_dtypes.float8_e4m3)                  # e4m3, RTNE
    # [slab, partition, d1, col]: per-partition-contiguous 4KB lines
    p8 = np.ascontiguousarray(
        e8.reshape(KC, 128, NHS, HS).transpose(2, 1, 0, 3)
    )

    in_maps = []
    for c in range(NCORES):
        items = _core_items(c)
        lr = np.concatenate(
            [p8[[i for i, _ in items]], p8[[j for _, j in items]]], axis=3
        )
        in_maps.append({"lr": lr})

    res = run_bass_kernel_spmd(nc, in_maps, list(range(NCORES)))
    LAST_RESULTS = res

    m = min(
        min(
            np.asarray(r["rm0"], np.float32).min(),
            np.asarray(r["rm1"], np.float32).min(),
            r["slots_o"].min(),
        )
        for r in res.results
    )

    blocks = np.empty((N_GRAPHS, G, G), np.float32)
    gph = HS // G  # graphs per half-slab = 8
    for c in range(NCORES):
        rows = [c, NHS - 1 - c]
        gids = [i * gph + k for i in rows for k in range(gph)]
        raw = res.results[c]["diag_out"]          # [128, GPC*G]
        for idx, g in enumerate(gids):
            blocks[g] = raw[:, idx * G : (idx + 1) * G]

    M = np.einsum("gii->gi", blocks).max()                 # global max (Cauchy-Schwarz)

    norm = (blocks - m) / (M - m + EPS)
    r = row_idx.astype(np.int64)
    cc = col_idx.astype(np.int64)
    out = norm[r >> 7, r & 127, cc & 127].astype(np.float32)
    return out
